# revision 2
# baseline (speedup 1.0000x reference)
"""GQA kernel v3 for Trainium2, sharded over 8 NeuronCores.

Core c = b*4 + h handles batch b, kv-head h (4 grouped q-heads).
Software-pipelined single program:
  prework: k/v/q0 projections (v PE-transposed into row-major va).
  8 stages (g-major over (g, ib)): scoresT[j,i] -> exp (ACT) -> AV in
    avT[d,i] orientation (va[j] stationary, et moving). q1..q3 projection
    matmuls are woven into PE slack slots of stages 1-5. Softmax
    denominator: et tiles accumulated on DVE+GpSimd as two chains,
    merged, partition-summed by gpsimd partition_all_reduce, fast
    reciprocal, normalize straight out of PSUM into attnT.
  avT psum alternates pools on stages 6-7 so boundary normalize never
    stalls AV; stage 8 runs in its own PSUM scope with the first half
    of phase 3 (i<1024 chunks) woven in as its filler.
  phase 3: outT[e,i] = WoT @ attnT, bf16 partials, host-summed.
"""

import math

import numpy as np

B = 2
N = 2048
E = 2048
HQ = 16
G = 4
HKV = 4
D = 128
P = 128
NB = N // 512
ET = E // P
JT = N // P
IB = N // 1024
SCALE = 1.0 / math.sqrt(D)
AV_LAG = 6
CHAIN_B_J = 8  # et_0..7 -> chain A, et_8..15 -> chain B (both DVE, bf16)

_CACHE: dict = {}


def _build_program():
    import concourse.bacc as bacc
    import concourse.tile as tile
    from concourse import bass_isa, mybir
    from concourse.masks import make_identity

    f32 = mybir.dt.float32
    bf16 = mybir.dt.bfloat16
    Alu = mybir.AluOpType
    nc = bacc.Bacc("TRN2", target_bir_lowering=False)

    xT_d = nc.dram_tensor("xT", [ET, P, N], bf16, kind="ExternalInput")
    wqT_d = nc.dram_tensor("wqT", [G, P, ET, D], bf16, kind="ExternalInput")
    wkT_d = nc.dram_tensor("wkT", [P, ET, D], bf16, kind="ExternalInput")
    wvT_d = nc.dram_tensor("wvT", [P, ET, D], bf16, kind="ExternalInput")
    woT_d = nc.dram_tensor("woT", [P, G, N], bf16, kind="ExternalInput")
    outT_d = nc.dram_tensor("outT", [ET, P, N], f32, kind="ExternalOutput")

    with tile.TileContext(nc) as tc:
        with tc.tile_pool(name="persist", bufs=1) as ps:
            ident = ps.tile([P, P], bf16, tag="ident")
            make_identity(nc, ident)

            wq_sb = ps.tile([P, G, ET, D], bf16, tag="wq_sb")
            wk_sb = ps.tile([P, ET, D], bf16, tag="wk_sb")
            wv_sb = ps.tile([P, ET, D], bf16, tag="wv_sb")
            wo_sb = ps.tile([P, G, N], bf16, tag="wo_sb")
            xts = [ps.tile([P, N], bf16, name=f"xt{e}", tag=f"xt{e}")
                   for e in range(ET)]
            kT = ps.tile([P, N], bf16, tag="kT")
            qT = [ps.tile([P, N], bf16, name=f"qT{g}", tag=f"qT{g}")
                  for g in range(G)]
            va = ps.tile([P, JT, D], bf16, tag="va")
            ets = [ps.tile([P, 1024], bf16, name=f"et{j}", tag=f"et{j}")
                   for j in range(JT)]
            attnT = [ps.tile([P, N], bf16, name=f"attnT{g}", tag=f"attnT{g}")
                     for g in range(G)]

            # ---- input DMAs (ordered so prework is fed promptly) ----
            nc.sync.dma_start(out=wk_sb[:], in_=wkT_d[:])
            for e in range(ET):
                nc.sync.dma_start(out=xts[e][:, 0:512], in_=xT_d[e, :, 0:512])
            nc.sync.dma_start(out=wq_sb[:, 0], in_=wqT_d[0])
            for e in range(ET):
                nc.sync.dma_start(out=xts[e][:, 512:1024],
                                  in_=xT_d[e, :, 512:1024])
            nc.sync.dma_start(out=wv_sb[:], in_=wvT_d[:])
            for nb in range(2, NB):
                sl = slice(nb * 512, (nb + 1) * 512)
                for e in range(ET):
                    nc.sync.dma_start(out=xts[e][:, sl], in_=xT_d[e, :, sl])
            for g in range(1, G):
                nc.sync.dma_start(out=wq_sb[:, g], in_=wqT_d[g])
            nc.sync.dma_start(out=wo_sb[:], in_=woT_d[:])

            with tc.tile_pool(name="sbsm", bufs=1) as sbsm:
                vTc = sbsm.tile([P, 512], bf16, tag="vTc", bufs=2)
                dvacc = sbsm.tile([P, 1024], bf16, tag="dvacc", bufs=2)
                dvaccB = sbsm.tile([P, 1024], bf16, tag="dvaccB", bufs=2)
                rbc = sbsm.tile([P, 1024], f32, tag="rbc", bufs=1)
                avsb = sbsm.tile([P, 1024], bf16, tag="avsb", bufs=2)
                rsbA = sbsm.tile([1, 512], f32, tag="rsbA", bufs=1)
                rsbB = sbsm.tile([1, 512], f32, tag="rsbB", bufs=1)
                onesf = sbsm.tile([P, 1], bf16, tag="onesf", bufs=1)
                nc.vector.memset(onesf[:], 1.0)

                def proj_mms(pool, w_ap, nb, dst, dst_sl):
                    """16 accumulation MMs + copy for one [128,512] chunk."""
                    pt = pool.tile([P, 512], f32, name="pt", tag="proj",
                                   bufs=2)
                    sl = slice(nb * 512, (nb + 1) * 512)
                    for e in range(ET):
                        yield lambda e=e, pt=pt, sl=sl: nc.tensor.matmul(
                            pt[:], w_ap(e), xts[e][:, sl],
                            start=(e == 0), stop=(e == ET - 1),
                        )
                    yield lambda pt=pt: nc.vector.tensor_copy(
                        dst[:, dst_sl], pt[:])

                def run_gen(gen):
                    for f in gen:
                        f()

                # ---------- prework: k, v (+ transposes), q0 ----------
                with tc.tile_pool(name="ppre", bufs=1, space="PSUM") as ppre, \
                     tc.tile_pool(name="ptr", bufs=2, space="PSUM") as ptr:
                    for nb in range(NB):
                        sl = slice(nb * 512, (nb + 1) * 512)
                        run_gen(proj_mms(
                            ppre, lambda e: wk_sb[:, e, :], nb, kT, sl))
                        run_gen(proj_mms(
                            ppre, lambda e: wq_sb[:, 0, e, :], nb, qT[0], sl))
                    for nb in range(NB):
                        run_gen(proj_mms(
                            ppre, lambda e: wv_sb[:, e, :], nb, vTc,
                            slice(0, 512)))
                        for c in range(4):
                            j = nb * 4 + c
                            tp = ptr.tile([P, P], bf16, name="tp", tag="tp")
                            nc.tensor.transpose(
                                tp[:], vTc[:, c * P:(c + 1) * P], ident[:])
                            nc.vector.tensor_copy(va[:, j, :], tp[:])

                fill: list = []

                def emit_fill(k):
                    while k > 0 and fill:
                        fill.pop()()
                        k -= 1

                stages = [(g, ib) for g in range(G) for ib in range(IB)]

                pending_norm = [None]

                def emit_stage(si, psc, pav, pf, fill_sched):
                    g, ib = stages[si]
                    isl = slice(ib * 1024, (ib + 1) * 1024)
                    avp = pav.tile([P, 1024], f32, name="avp",
                                   tag="avT", bufs=1)

                    def av_mm(j):
                        for h in range(2):
                            hs = slice(h * 512, (h + 1) * 512)
                            nc.tensor.matmul(
                                avp[:, hs], va[:, j, :], ets[j][:, hs],
                                start=(j == 0), stop=(j == JT - 1),
                            )

                    for j in range(JT):
                        sps = psc.tile([P, 1024], f32, name="sps",
                                       tag="sps", bufs=2)
                        for h in range(2):
                            nc.tensor.matmul(
                                sps[:, h * 512:(h + 1) * 512],
                                kT[:, j * P:(j + 1) * P],
                                qT[g][:, ib * 1024 + h * 512:
                                       ib * 1024 + (h + 1) * 512],
                                start=True, stop=True,
                            )
                        nc.scalar.activation(
                            ets[j][:], sps[:],
                            mybir.ActivationFunctionType.Exp, scale=SCALE,
                        )
                        # two bf16 denominator chains on DVE
                        if j == 0:
                            nc.vector.tensor_copy(dvacc[:], ets[0][:])
                        elif j < CHAIN_B_J:
                            nc.vector.tensor_tensor(
                                dvacc[:], dvacc[:], ets[j][:], Alu.add)
                        elif j == CHAIN_B_J:
                            nc.vector.tensor_copy(dvaccB[:], ets[j][:])
                        else:
                            nc.vector.tensor_tensor(
                                dvaccB[:], dvaccB[:], ets[j][:], Alu.add)
                        if j >= AV_LAG:
                            av_mm(j - AV_LAG)
                        if j == 3 and pending_norm[0] is not None:
                            pending_norm[0]()
                            pending_norm[0] = None
                        emit_fill(fill_sched(j))
                    for j in range(JT - AV_LAG, JT):
                        av_mm(j)
                    # free avp early: unnormalized AV sums to SBUF (bf16)
                    nc.vector.tensor_copy(avsb[:], avp[:])
                    # partition-sum both chains via PE ones-matmuls
                    dns = []
                    for h in range(2):
                        hs = slice(h * 512, (h + 1) * 512)
                        dn = pf.tile([P, 512], f32, name="dn", tag="proj",
                                     bufs=2)
                        nc.tensor.matmul(
                            dn[0:1, :], onesf[:], dvacc[:, hs],
                            start=True, stop=False,
                        )
                        nc.tensor.matmul(
                            dn[0:1, :], onesf[:], dvaccB[:, hs],
                            start=False, stop=True,
                        )
                        dns.append(dn)
                    nc.vector.reciprocal_approx_fast(rsbA[:], dns[0][0:1, :])
                    nc.vector.reciprocal_approx_fast(rsbB[:], dns[1][0:1, :])
                    nc.gpsimd.partition_broadcast(rbc[:, 0:512], rsbA[:])
                    nc.gpsimd.partition_broadcast(rbc[:, 512:1024], rsbB[:])

                    def norm(g=g, isl=isl, av=avsb):
                        nc.vector.tensor_mul(attnT[g][:, isl], av[:], rbc[:])
                    pending_norm[0] = norm

                def ph3_chunk(pool, eo, c):
                    esl = slice(eo * P, (eo + 1) * P)
                    csl = slice(c * 512, (c + 1) * 512)
                    po = pool.tile([P, 512], f32, name="po", tag="proj",
                                   bufs=2)
                    for g in range(G):
                        yield lambda g=g, po=po: nc.tensor.matmul(
                            po[:], wo_sb[:, g, esl], attnT[g][:, csl],
                            start=(g == 0), stop=(g == G - 1),
                        )

                    def fin(po=po):
                        o = sbsm.tile([P, 512], f32, name="o", tag="ot",
                                      bufs=3)
                        nc.vector.tensor_copy(o[:], po[:])
                        nc.sync.dma_start(out=outT_d[eo, :, csl], in_=o[:])
                    yield fin

                # ---------- stages + phase 3 (uniform pools) ----------
                with tc.tile_pool(name="psc", bufs=1, space="PSUM") as psc, \
                     tc.tile_pool(name="pav", bufs=1, space="PSUM") as pav, \
                     tc.tile_pool(name="pf", bufs=1, space="PSUM") as pf:
                    for g in range(1, G):
                        for nb in range(NB):
                            sl = slice(nb * 512, (nb + 1) * 512)
                            fill.extend(proj_mms(
                                pf, lambda e, g=g: wq_sb[:, g, e, :],
                                nb, qT[g], sl))
                    fill.reverse()
                    for si in range(7):
                        emit_stage(si, psc, pav, pf, lambda j: 3)
                    emit_fill(len(fill))
                    for eo in range(ET):
                        for c in (0, 1):
                            fill.extend(ph3_chunk(pf, eo, c))
                    fill.reverse()
                    emit_stage(7, psc, pav, pf,
                               lambda j: 5 if j >= 6 else 0)
                    pending_norm[0]()
                    pending_norm[0] = None
                    emit_fill(len(fill))
                    for eo in range(ET):
                        for c in (2, 3):
                            run_gen(ph3_chunk(pf, eo, c))
    nc.finalize()
    return nc


def _get_program():
    if "nc" not in _CACHE:
        _CACHE["nc"] = _build_program()
    return _CACHE["nc"]


def _make_in_maps(x, Wq, Wk, Wv, Wo):
    import ml_dtypes

    bf = ml_dtypes.bfloat16
    FQ = G * D

    def wtile(w):  # [rows, E] -> [P, ET, rows] tiled on partition
        r = w.shape[0]
        return np.ascontiguousarray(
            w.T.reshape(ET, P, r).transpose(1, 0, 2)
        ).astype(bf)

    xT = [
        np.ascontiguousarray(x[b].T).astype(bf).reshape(ET, P, N)
        for b in range(B)
    ]
    in_maps = []
    for c in range(8):
        b, h = c // HKV, c % HKV
        wq = Wq[h * FQ:(h + 1) * FQ, :]  # [512, E]
        wqt = np.stack([wtile(wq[g * D:(g + 1) * D, :]) for g in range(G)])
        wo = Wo[:, h * FQ:(h + 1) * FQ].T  # [FQ, E]
        in_maps.append({
            "xT": xT[b],
            "wqT": wqt,
            "wkT": wtile(Wk[h * D:(h + 1) * D, :]),
            "wvT": wtile(Wv[h * D:(h + 1) * D, :]),
            "woT": np.ascontiguousarray(
                wo.reshape(G, P, N).transpose(1, 0, 2)
            ).astype(bf),
        })
    return in_maps


def run_spmd(in_maps, trace=False, **kw):
    from concourse.bass_utils import run_bass_kernel_spmd

    nc = _get_program()
    return run_bass_kernel_spmd(nc, in_maps, list(range(8)), trace=trace, **kw)


def kernel(x, Wq, Wk, Wv, Wo, next_token_only=0, **_ignored):
    x = np.asarray(x, dtype=np.float32)
    Wq = np.asarray(Wq, dtype=np.float32)
    Wk = np.asarray(Wk, dtype=np.float32)
    Wv = np.asarray(Wv, dtype=np.float32)
    Wo = np.asarray(Wo, dtype=np.float32)

    res = run_spmd(_make_in_maps(x, Wq, Wk, Wv, Wo))
    outs = [
        np.asarray(r["outT"], dtype=np.float32).reshape(E, N)
        for r in res.results
    ]
    full = np.empty((B, N, E), np.float32)
    for b in range(B):
        acc = outs[b * HKV].copy()
        for h in range(1, HKV):
            acc += outs[b * HKV + h]
        full[b] = acc.T
    return full


# revision 3
# speedup vs baseline: 1.0592x; 1.0592x over previous
"""GQA kernel v3 for Trainium2, sharded over 8 NeuronCores.

Core c = b*4 + h handles batch b, kv-head h (4 grouped q-heads).
Software-pipelined single program:
  prework: k/v/q0 projections (v PE-transposed into row-major va).
  8 stages (g-major over (g, ib)): scoresT[j,i] -> exp (ACT) -> AV in
    avT[d,i] orientation (va[j] stationary, et moving). q1..q3 projection
    matmuls are woven into PE slack slots of stages 1-5. Softmax
    denominator: et tiles accumulated on DVE+GpSimd as two chains,
    merged, partition-summed by gpsimd partition_all_reduce, fast
    reciprocal, normalize straight out of PSUM into attnT.
  avT psum alternates pools on stages 6-7 so boundary normalize never
    stalls AV; stage 8 runs in its own PSUM scope with the first half
    of phase 3 (i<1024 chunks) woven in as its filler.
  phase 3: outT[e,i] = WoT @ attnT, bf16 partials, host-summed.
"""

import math

import numpy as np

B = 2
N = 2048
E = 2048
HQ = 16
G = 4
HKV = 4
D = 128
P = 128
NB = N // 512
ET = E // P
JT = N // P
IB = N // 1024
SCALE = 1.0 / math.sqrt(D)
AV_LAG = 6
CHAIN_B_J = 8  # et_0..7 -> chain A, et_8..15 -> chain B (both DVE, bf16)

_CACHE: dict = {}


def _build_program():
    import concourse.bacc as bacc
    import concourse.tile as tile
    from concourse import bass_isa, mybir
    from concourse.masks import make_identity

    f32 = mybir.dt.float32
    bf16 = mybir.dt.bfloat16
    Alu = mybir.AluOpType
    nc = bacc.Bacc("TRN2", target_bir_lowering=False)

    xT_d = nc.dram_tensor("xT", [ET, P, N], bf16, kind="ExternalInput")
    wqT_d = nc.dram_tensor("wqT", [G, P, ET, D], bf16, kind="ExternalInput")
    wkT_d = nc.dram_tensor("wkT", [P, ET, D], bf16, kind="ExternalInput")
    wvT_d = nc.dram_tensor("wvT", [P, ET, D], bf16, kind="ExternalInput")
    woT_d = nc.dram_tensor("woT", [P, G, N], bf16, kind="ExternalInput")
    outT_d = nc.dram_tensor("outT", [ET, P, N], bf16, kind="ExternalOutput")

    with tile.TileContext(nc) as tc:
        with tc.tile_pool(name="persist", bufs=1) as ps:
            ident = ps.tile([P, P], bf16, tag="ident")
            make_identity(nc, ident)

            wq_sb = ps.tile([P, G, ET, D], bf16, tag="wq_sb")
            wk_sb = ps.tile([P, ET, D], bf16, tag="wk_sb")
            wv_sb = ps.tile([P, ET, D], bf16, tag="wv_sb")
            wo_sb = ps.tile([P, G, N], bf16, tag="wo_sb")
            xts = [ps.tile([P, N], bf16, name=f"xt{e}", tag=f"xt{e}")
                   for e in range(ET)]
            kT = ps.tile([P, N], bf16, tag="kT")
            qT = [ps.tile([P, N], bf16, name=f"qT{g}", tag=f"qT{g}")
                  for g in range(G)]
            va = ps.tile([P, JT, D], bf16, tag="va")
            ets = [ps.tile([P, 1024], bf16, name=f"et{j}", tag=f"et{j}")
                   for j in range(JT)]
            attnT = [ps.tile([P, N], bf16, name=f"attnT{g}", tag=f"attnT{g}")
                     for g in range(G)]

            # ---- input DMAs (ordered so prework is fed promptly) ----
            nc.sync.dma_start(out=wk_sb[:], in_=wkT_d[:])
            for e in range(ET):
                nc.sync.dma_start(out=xts[e][:, 0:512], in_=xT_d[e, :, 0:512])
            nc.sync.dma_start(out=wq_sb[:, 0], in_=wqT_d[0])
            for e in range(ET):
                nc.sync.dma_start(out=xts[e][:, 512:1024],
                                  in_=xT_d[e, :, 512:1024])
            nc.sync.dma_start(out=wv_sb[:], in_=wvT_d[:])
            for nb in range(2, NB):
                sl = slice(nb * 512, (nb + 1) * 512)
                for e in range(ET):
                    nc.sync.dma_start(out=xts[e][:, sl], in_=xT_d[e, :, sl])
            for g in range(1, G):
                nc.sync.dma_start(out=wq_sb[:, g], in_=wqT_d[g])
            nc.sync.dma_start(out=wo_sb[:], in_=woT_d[:])

            with tc.tile_pool(name="sbsm", bufs=1) as sbsm:
                vTc = sbsm.tile([P, 512], bf16, tag="vTc", bufs=2)
                dvacc = sbsm.tile([P, 1024], bf16, tag="dvacc", bufs=2)
                dvaccB = sbsm.tile([P, 1024], bf16, tag="dvaccB", bufs=2)
                rbc = sbsm.tile([P, 1024], f32, tag="rbc", bufs=1)
                avsb = sbsm.tile([P, 1024], bf16, tag="avsb", bufs=2)
                rsbA = sbsm.tile([1, 512], f32, tag="rsbA", bufs=1)
                rsbB = sbsm.tile([1, 512], f32, tag="rsbB", bufs=1)
                onesf = sbsm.tile([P, 1], bf16, tag="onesf", bufs=1)
                nc.vector.memset(onesf[:], 1.0)

                def proj_mms(pool, w_ap, nb, dst, dst_sl):
                    """16 accumulation MMs + copy for one [128,512] chunk."""
                    pt = pool.tile([P, 512], f32, name="pt", tag="proj",
                                   bufs=2)
                    sl = slice(nb * 512, (nb + 1) * 512)
                    for e in range(ET):
                        yield lambda e=e, pt=pt, sl=sl: nc.tensor.matmul(
                            pt[:], w_ap(e), xts[e][:, sl],
                            start=(e == 0), stop=(e == ET - 1),
                        )
                    yield lambda pt=pt: nc.vector.tensor_copy(
                        dst[:, dst_sl], pt[:])

                def run_gen(gen):
                    for f in gen:
                        f()

                # ---------- prework: k, v (+ transposes), q0 ----------
                with tc.tile_pool(name="ppre", bufs=1, space="PSUM") as ppre, \
                     tc.tile_pool(name="ptr", bufs=2, space="PSUM") as ptr:
                    for nb in range(NB):
                        sl = slice(nb * 512, (nb + 1) * 512)
                        run_gen(proj_mms(
                            ppre, lambda e: wk_sb[:, e, :], nb, kT, sl))
                        run_gen(proj_mms(
                            ppre, lambda e: wq_sb[:, 0, e, :], nb, qT[0], sl))
                    for nb in range(NB):
                        run_gen(proj_mms(
                            ppre, lambda e: wv_sb[:, e, :], nb, vTc,
                            slice(0, 512)))
                        for c in range(4):
                            j = nb * 4 + c
                            tp = ptr.tile([P, P], bf16, name="tp", tag="tp")
                            nc.tensor.transpose(
                                tp[:], vTc[:, c * P:(c + 1) * P], ident[:])
                            nc.vector.tensor_copy(va[:, j, :], tp[:])

                fill: list = []

                def emit_fill(k):
                    while k > 0 and fill:
                        fill.pop()()
                        k -= 1

                stages = [(g, ib) for g in range(G) for ib in range(IB)]

                pending_norm = [None]

                def emit_stage(si, psc, pav, pf, fill_sched):
                    g, ib = stages[si]
                    isl = slice(ib * 1024, (ib + 1) * 1024)
                    avp = pav.tile([P, 1024], f32, name="avp",
                                   tag="avT", bufs=1)

                    def av_mm(j):
                        for h in range(2):
                            hs = slice(h * 512, (h + 1) * 512)
                            nc.tensor.matmul(
                                avp[:, hs], va[:, j, :], ets[j][:, hs],
                                start=(j == 0), stop=(j == JT - 1),
                            )

                    for j in range(JT):
                        sps = psc.tile([P, 1024], f32, name="sps",
                                       tag="sps", bufs=2)
                        for h in range(2):
                            nc.tensor.matmul(
                                sps[:, h * 512:(h + 1) * 512],
                                kT[:, j * P:(j + 1) * P],
                                qT[g][:, ib * 1024 + h * 512:
                                       ib * 1024 + (h + 1) * 512],
                                start=True, stop=True,
                            )
                        nc.scalar.activation(
                            ets[j][:], sps[:],
                            mybir.ActivationFunctionType.Exp, scale=SCALE,
                        )
                        # two bf16 denominator chains on DVE
                        if j == 0:
                            nc.vector.tensor_copy(dvacc[:], ets[0][:])
                        elif j < CHAIN_B_J:
                            nc.vector.tensor_tensor(
                                dvacc[:], dvacc[:], ets[j][:], Alu.add)
                        elif j == CHAIN_B_J:
                            nc.vector.tensor_copy(dvaccB[:], ets[j][:])
                        else:
                            nc.vector.tensor_tensor(
                                dvaccB[:], dvaccB[:], ets[j][:], Alu.add)
                        if j >= AV_LAG:
                            av_mm(j - AV_LAG)
                        if j == 3 and pending_norm[0] is not None:
                            pending_norm[0]()
                            pending_norm[0] = None
                        emit_fill(fill_sched(j))
                    for j in range(JT - AV_LAG, JT):
                        av_mm(j)
                    # free avp early: unnormalized AV sums to SBUF (bf16)
                    nc.vector.tensor_copy(avsb[:], avp[:])
                    # partition-sum both chains via PE ones-matmuls
                    dns = []
                    for h in range(2):
                        hs = slice(h * 512, (h + 1) * 512)
                        dn = pf.tile([P, 512], f32, name="dn", tag="proj",
                                     bufs=2)
                        nc.tensor.matmul(
                            dn[0:1, :], onesf[:], dvacc[:, hs],
                            start=True, stop=False,
                        )
                        nc.tensor.matmul(
                            dn[0:1, :], onesf[:], dvaccB[:, hs],
                            start=False, stop=True,
                        )
                        dns.append(dn)
                    nc.vector.reciprocal_approx_fast(rsbA[:], dns[0][0:1, :])
                    nc.vector.reciprocal_approx_fast(rsbB[:], dns[1][0:1, :])
                    nc.gpsimd.partition_broadcast(rbc[:, 0:512], rsbA[:])
                    nc.gpsimd.partition_broadcast(rbc[:, 512:1024], rsbB[:])

                    def norm(g=g, isl=isl, av=avsb):
                        nc.vector.tensor_mul(attnT[g][:, isl], av[:], rbc[:])
                    pending_norm[0] = norm

                def ph3_chunk(pool, eo, c):
                    esl = slice(eo * P, (eo + 1) * P)
                    csl = slice(c * 512, (c + 1) * 512)
                    po = pool.tile([P, 512], f32, name="po", tag="proj",
                                   bufs=2)
                    for g in range(G):
                        yield lambda g=g, po=po: nc.tensor.matmul(
                            po[:], wo_sb[:, g, esl], attnT[g][:, csl],
                            start=(g == 0), stop=(g == G - 1),
                        )

                    def fin(po=po):
                        o = sbsm.tile([P, 512], bf16, name="o", tag="ot",
                                      bufs=3)
                        nc.vector.tensor_copy(o[:], po[:])
                        nc.sync.dma_start(out=outT_d[eo, :, csl], in_=o[:])
                    yield fin

                # ---------- stages + phase 3 (uniform pools) ----------
                with tc.tile_pool(name="psc", bufs=1, space="PSUM") as psc, \
                     tc.tile_pool(name="pav", bufs=1, space="PSUM") as pav, \
                     tc.tile_pool(name="pf", bufs=1, space="PSUM") as pf:
                    for g in range(1, G):
                        for nb in range(NB):
                            sl = slice(nb * 512, (nb + 1) * 512)
                            fill.extend(proj_mms(
                                pf, lambda e, g=g: wq_sb[:, g, e, :],
                                nb, qT[g], sl))
                    fill.reverse()
                    for si in range(7):
                        emit_stage(si, psc, pav, pf, lambda j: 3)
                    emit_fill(len(fill))
                    for eo in range(ET):
                        for c in (0, 1):
                            fill.extend(ph3_chunk(pf, eo, c))
                    fill.reverse()
                    emit_stage(7, psc, pav, pf,
                               lambda j: 5 if j >= 6 else 0)
                    pending_norm[0]()
                    pending_norm[0] = None
                    emit_fill(len(fill))
                    for eo in range(ET):
                        for c in (2, 3):
                            run_gen(ph3_chunk(pf, eo, c))
    nc.finalize()
    return nc


def _get_program():
    if "nc" not in _CACHE:
        _CACHE["nc"] = _build_program()
    return _CACHE["nc"]


def _make_in_maps(x, Wq, Wk, Wv, Wo):
    import ml_dtypes

    bf = ml_dtypes.bfloat16
    FQ = G * D

    def wtile(w):  # [rows, E] -> [P, ET, rows] tiled on partition
        r = w.shape[0]
        return np.ascontiguousarray(
            w.T.reshape(ET, P, r).transpose(1, 0, 2)
        ).astype(bf)

    xT = [
        np.ascontiguousarray(x[b].T).astype(bf).reshape(ET, P, N)
        for b in range(B)
    ]
    in_maps = []
    for c in range(8):
        b, h = c // HKV, c % HKV
        wq = Wq[h * FQ:(h + 1) * FQ, :]  # [512, E]
        wqt = np.stack([wtile(wq[g * D:(g + 1) * D, :]) for g in range(G)])
        wo = Wo[:, h * FQ:(h + 1) * FQ].T  # [FQ, E]
        in_maps.append({
            "xT": xT[b],
            "wqT": wqt,
            "wkT": wtile(Wk[h * D:(h + 1) * D, :]),
            "wvT": wtile(Wv[h * D:(h + 1) * D, :]),
            "woT": np.ascontiguousarray(
                wo.reshape(G, P, N).transpose(1, 0, 2)
            ).astype(bf),
        })
    return in_maps


def run_spmd(in_maps, trace=False, **kw):
    from concourse.bass_utils import run_bass_kernel_spmd

    nc = _get_program()
    return run_bass_kernel_spmd(nc, in_maps, list(range(8)), trace=trace, **kw)


def kernel(x, Wq, Wk, Wv, Wo, next_token_only=0, **_ignored):
    x = np.asarray(x, dtype=np.float32)
    Wq = np.asarray(Wq, dtype=np.float32)
    Wk = np.asarray(Wk, dtype=np.float32)
    Wv = np.asarray(Wv, dtype=np.float32)
    Wo = np.asarray(Wo, dtype=np.float32)

    res = run_spmd(_make_in_maps(x, Wq, Wk, Wv, Wo))
    outs = [
        np.asarray(r["outT"], dtype=np.float32).reshape(E, N)
        for r in res.results
    ]
    full = np.empty((B, N, E), np.float32)
    for b in range(B):
        acc = outs[b * HKV].copy()
        for h in range(1, HKV):
            acc += outs[b * HKV + h]
        full[b] = acc.T
    return full
